# revision 24
# baseline (speedup 1.0000x reference)
"""Trainium2 Bass kernel for CustomGraphConv message passing.

reference computation:
    msg  = einsum('eoi,ei->eo', W, x[src])          # per-edge matvec
    aggr = segment_sum(msg, dst, num_segments=N)     # scatter-add
    out  = relu(aggr + bias)

Strategy (8 NeuronCores):
  - Partition OUTPUT nodes across cores: core c owns dst in [c*N/8, (c+1)*N/8).
    Edges are routed to the core owning their destination -> no all-reduce.
  - Within a core, edges are binned by destination block of 128 nodes
    (98 blocks x 128 = 12544 >= 12500), each block padded to a uniform
    tile count T (pad edges have zero weights -> contribute nothing).
  - Per 128-edge tile (one edge per SBUF partition):
      S[e, n]     = (dst_local[e] == n)        one-hot      (DVE is_equal)
      Q[e, (o,i)] = W[e,o,i] * xj[e,i]         products     (DVE mult)
      PSUM[n,(o,i)] += S.T @ Q                 scatter-add  (PE matmul)
    After T tiles: out[n,o] = relu(sum_i PSUM[n,(o,i)] + bias[o])
  - x[src] is gathered on host and streamed densely (adds ~6% traffic vs W).
"""

import os
import sys
import numpy as np

sys.path.insert(0, "/opt/trn_rl_repo")

_LAST_RUN_INFO = {}

N_CORES = 8
BLK = 128          # nodes per destination block (= one-hot window / PSUM rows)
G = 4              # tiles per batched DVE op / W-DMA
IN_C = 16
OUT_C = 16


def _install_ntff_hook():
    """Provide antenv.axon_hooks if the image lacks it (profiling only)."""
    import importlib.util
    import types
    import contextlib
    import ctypes

    if "antenv.axon_hooks" in sys.modules:
        return
    try:
        import antenv.axon_hooks  # noqa: F401
        return
    except ImportError:
        pass

    mod = types.ModuleType("antenv.axon_hooks")
    mod._hook = None
    mod._tried = False

    def set_axon_ntff_profile_hook(h):
        mod._hook = h

    def _via_ctypes(so_path):
        lib = ctypes.CDLL(so_path)
        if not hasattr(lib, "axon_start_nrt_profile"):
            return None
        lib.axon_start_nrt_profile.argtypes = [
            ctypes.POINTER(ctypes.c_int64),
            ctypes.c_size_t,
        ]
        lib.axon_start_nrt_profile.restype = ctypes.c_int64
        lib.axon_stop_nrt_profile.argtypes = [ctypes.c_char_p]
        lib.axon_stop_nrt_profile.restype = ctypes.c_int64

        @contextlib.contextmanager
        def _hook_cm(output_dir, device_ids):
            import jax

            jax.devices()
            if device_ids:
                ids = (ctypes.c_int64 * len(device_ids))(*device_ids)
                rc = lib.axon_start_nrt_profile(ids, len(device_ids))
            else:
                rc = lib.axon_start_nrt_profile(None, 0)
            if rc != 0:
                raise RuntimeError(f"axon_start_nrt_profile rc={rc}")
            try:
                yield
            finally:
                n = lib.axon_stop_nrt_profile(str(output_dir).encode())
                print(f"profile: {n} file(s) written to {output_dir}", file=sys.stderr)

        return _hook_cm

    def get_axon_ntff_profile_hook():
        if mod._hook is None and not mod._tried:
            mod._tried = True
            so = os.environ.get("AXON_PJRT_SO", "/opt/axon/libaxon_pjrt.so")
            if os.path.exists(so):
                try:
                    mod._hook = _via_ctypes(so)
                except OSError:
                    mod._hook = None
        return mod._hook

    mod.set_axon_ntff_profile_hook = set_axon_ntff_profile_hook
    mod.get_axon_ntff_profile_hook = get_axon_ntff_profile_hook
    sys.modules["antenv.axon_hooks"] = mod


def _build_bass(tiles_total, nblk, t_per_blk, stream_fp16):
    import concourse.bass as bass
    import concourse.bacc as bacc
    import concourse.tile as tile
    import concourse.mybir as mybir

    f32 = mybir.dt.float32
    f16 = mybir.dt.bfloat16
    sdt = f16 if stream_fp16 else f32   # dtype of W / xj streams

    nc = bacc.Bacc("TRN2", target_bir_lowering=False, debug=False,
                   num_devices=N_CORES)

    T = t_per_blk
    H = T // 2                       # tiles per local_scatter call (T even)
    NI = H + (H % 2)                 # num_idxs (padded even)
    w_d = nc.dram_tensor("w", [128, tiles_total, 256], sdt, kind="ExternalInput")
    xj_d = nc.dram_tensor("xj", [128, tiles_total, IN_C], sdt, kind="ExternalInput")
    sx_d = nc.dram_tensor("sidx", [128, nblk, 2, NI], mybir.dt.int16,
                          kind="ExternalInput")
    bias_d = nc.dram_tensor("biasb", [128, OUT_C], f32, kind="ExternalInput")
    out_d = nc.dram_tensor("out", [nblk, 128, OUT_C], f32, kind="ExternalOutput")

    with tile.TileContext(nc) as tc:
        with (
            tc.tile_pool(name="wpool", bufs=6) as wpool,
            tc.tile_pool(name="xpool", bufs=4) as xpool,
            tc.tile_pool(name="dpool", bufs=8) as dpool,
            tc.tile_pool(name="spool", bufs=6) as spool,
            tc.tile_pool(name="qpool", bufs=4) as qpool,
            tc.tile_pool(name="opool", bufs=3) as opool,
            tc.tile_pool(name="cpool", bufs=1) as cpool,
            tc.tile_pool(name="psum", bufs=4, space="PSUM") as psum_pool,
        ):
            bias_t = cpool.tile([128, OUT_C], f32, tag="bias")
            nc.sync.dma_start(bias_t[:], bias_d[:])
            ones_t = cpool.tile([128, NI], f16, tag="ones")
            nc.vector.memset(ones_t[:], 1.0)

            ps_tiles = {}

            def emit_front(b):
                ps = psum_pool.tile([128, 256], f32)
                ps_tiles[b] = ps
                base = b * T
                # one DMA per stream per block (sequencer-issue amortized)
                wt = wpool.tile([128, T, 256], sdt, tag="wt")
                half = T // 2
                nc.sync.dma_start(wt[:, :half, :], w_d[:, base:base + half, :])
                nc.sync.dma_start(wt[:, half:, :],
                                  w_d[:, base + half:base + T, :])
                xt = xpool.tile([128, T, IN_C], sdt, tag="xt")
                nc.scalar.dma_start(xt[:], xj_d[:, base:base + T, :])
                sx = dpool.tile([128, 2, NI], mybir.dt.int16, tag="sx")
                nc.scalar.dma_start(sx[:], sx_d[:, b, :, :])

                # one-hot S built by GpSimd local scatter (zero + ones)
                st = spool.tile([128, T, BLK], f16, tag="st")
                for h in range(2):
                    nc.gpsimd.local_scatter(
                        st[:, h * H:(h + 1) * H, :].rearrange(
                            "p t n -> p (t n)"),
                        ones_t[:],
                        sx[:, h, :],
                        channels=128,
                        num_elems=H * BLK,
                        num_idxs=NI,
                    )
                qt = qpool.tile([128, T, OUT_C, IN_C], f16, tag="qt")
                nc.vector.tensor_tensor(
                    qt[:],
                    wt[:].rearrange("p g (o i) -> p g o i", i=IN_C),
                    xt[:].unsqueeze(2).broadcast_to([128, T, OUT_C, IN_C]),
                    op=mybir.AluOpType.mult,
                )
                for k in range(T):
                    nc.tensor.matmul(
                        ps[:],
                        st[:, k, :],
                        qt[:, k, :, :],
                        start=(k == 0),
                        stop=(k == T - 1),
                    )

            def emit_back(b):
                ps = ps_tiles.pop(b)
                ot = opool.tile([128, OUT_C], f32, tag="ot")
                nc.vector.tensor_reduce(
                    ot[:],
                    ps[:].rearrange("p (o i) -> p o i", i=IN_C),
                    axis=mybir.AxisListType.X,
                    op=mybir.AluOpType.add,
                )
                ob = opool.tile([128, OUT_C], f32, tag="ob")
                nc.vector.tensor_tensor(
                    ob[:], ot[:], bias_t[:], op=mybir.AluOpType.add)
                orl = opool.tile([128, OUT_C], f32, tag="orl")
                nc.scalar.activation(
                    orl[:], ob[:], mybir.ActivationFunctionType.Relu)
                nc.sync.dma_start(out_d[b], orl[:])

            # software pipeline: block b's PSUM eviction is emitted after
            # block b+1's front phase, so the DVE never stalls on the PE
            # finishing the current block (in-order engine queues).
            for b in range(nblk):
                emit_front(b)
                if b >= 1:
                    emit_back(b - 1)
            emit_back(nblk - 1)

    nc.compile()
    return nc


def kernel(x, edge_index, edge_attr, weights_matrices, bias,
           input_size, output_size, **_unused):
    _install_ntff_hook()
    import ml_dtypes

    stream_fp16 = bool(int(os.environ.get("GNN_STREAM_FP16", "1")))
    import ml_dtypes
    sdt_np = ml_dtypes.bfloat16 if stream_fp16 else np.float32

    x = np.asarray(x, dtype=np.float32)
    edge_index = np.asarray(edge_index)
    W = np.asarray(weights_matrices, dtype=np.float32)
    bias = np.asarray(bias, dtype=np.float32)

    N = x.shape[0]
    E = edge_index.shape[1]
    n_per_core = (N + N_CORES - 1) // N_CORES          # 12500
    nblk = (n_per_core + BLK - 1) // BLK               # 98

    src = edge_index[0].astype(np.int64)
    dst = edge_index[1].astype(np.int64)

    core = dst // n_per_core
    local = dst - core * n_per_core
    blk = local // BLK
    dstl = (local - blk * BLK).astype(np.float32)       # in [0,128)

    # group edges by (core, block)
    key = core * nblk + blk
    order = np.argsort(key, kind="stable")
    key_sorted = key[order]
    counts = np.bincount(key_sorted, minlength=N_CORES * nblk)
    t_per_blk = int((counts.max() + BLK - 1) // BLK)    # uniform tiles/block
    t_per_blk += t_per_blk % 2                          # even (scatter halves)
    tiles_total = nblk * t_per_blk
    epc = tiles_total * BLK                             # padded edges per core

    # slot position of each sorted edge inside its (core, block) bucket
    group_start = np.zeros(N_CORES * nblk + 1, dtype=np.int64)
    np.cumsum(counts, out=group_start[1:])
    within = np.arange(E, dtype=np.int64) - group_start[key_sorted]
    core_s = key_sorted // nblk
    blk_s = key_sorted - core_s * nblk
    pos = blk_s * (t_per_blk * BLK) + within            # slot within core

    # perm[c, slot] = original edge id, -1 for padding
    perm = np.full((N_CORES, epc), -1, dtype=np.int64)
    perm[core_s, pos] = order

    pad_mask = perm < 0
    perm_c = np.where(pad_mask, 0, perm)

    # build per-core streams; layout [cores, 128 partitions, tiles, ...]
    # edge slot s -> tile s // 128, partition s % 128
    def to_tiles(a):
        # a: [N_CORES, epc, F] -> [N_CORES, 128, tiles_total, F]
        F = a.shape[-1]
        return np.ascontiguousarray(
            a.reshape(N_CORES, tiles_total, BLK, F).transpose(0, 2, 1, 3))

    Wf = W.reshape(E, 256)
    w_perm = Wf[perm_c].astype(sdt_np)
    w_perm[pad_mask] = 0.0
    w_perm = to_tiles(w_perm)

    xj = x[src[perm_c]].astype(sdt_np)
    xj = to_tiles(xj)

    # scatter indices for the one-hot build:
    # edge slot s -> partition s%128, tile s//128; tile -> block b = tile//T,
    # k = tile%T, half h = k//H, idx value = (k%H)*BLK + dst_local  (-1 = pad)
    T = t_per_blk
    H = T // 2
    NI = H + (H % 2)
    s_arr = np.arange(epc, dtype=np.int64)
    p_arr = s_arr % BLK
    tile_arr = s_arr // BLK
    b_arr = tile_arr // T
    k_arr = tile_arr % T
    h_arr = k_arr // H
    kih_arr = k_arr % H
    dl_perm = dstl[perm_c].astype(np.int64)             # [N_CORES, epc]
    val = (kih_arr[None, :] * BLK + dl_perm).astype(np.int16)
    val[pad_mask] = -1
    sidx = np.full((N_CORES, 128, nblk, 2, NI), -1, dtype=np.int16)
    c_idx = np.repeat(np.arange(N_CORES), epc)
    sidx[c_idx, np.tile(p_arr, N_CORES), np.tile(b_arr, N_CORES),
         np.tile(h_arr, N_CORES), np.tile(kih_arr, N_CORES)] = val.ravel()

    bias_b = np.broadcast_to(bias, (128, OUT_C)).astype(np.float32)

    from concourse.bass_utils import run_bass_kernel_spmd

    nc = _build_bass(tiles_total, nblk, t_per_blk, stream_fp16)

    in_maps = [
        {
            "w": np.ascontiguousarray(w_perm[c]),
            "xj": np.ascontiguousarray(xj[c]),
            "sidx": np.ascontiguousarray(sidx[c]),
            "biasb": bias_b,
        }
        for c in range(N_CORES)
    ]

    trace = bool(int(os.environ.get("GNN_TRACE", "0")))
    res = run_bass_kernel_spmd(
        nc, in_maps, core_ids=list(range(N_CORES)), trace=trace)

    _LAST_RUN_INFO.clear()
    _LAST_RUN_INFO.update(
        exec_time_ns=res.exec_time_ns,
        mean_exec_time_ns=res.mean_exec_time_ns,
        tiles_total=tiles_total,
        t_per_blk=t_per_blk,
        profile_json=res.profile_json,
        instructions_and_trace=res.instructions_and_trace,
    )

    out = np.concatenate(
        [res.results[c]["out"].reshape(nblk * BLK, OUT_C)[:n_per_core]
         for c in range(N_CORES)], axis=0)
    return out[:N]


# revision 26
# speedup vs baseline: 1.1524x; 1.1524x over previous
"""Trainium2 Bass kernel for CustomGraphConv message passing.

reference computation:
    msg  = einsum('eoi,ei->eo', W, x[src])          # per-edge matvec
    aggr = segment_sum(msg, dst, num_segments=N)     # scatter-add
    out  = relu(aggr + bias)

Strategy (8 NeuronCores):
  - Partition OUTPUT nodes across cores: core c owns dst in [c*N/8, (c+1)*N/8).
    Edges are routed to the core owning their destination -> no all-reduce.
  - Within a core, edges are binned by destination block of 128 nodes
    (98 blocks x 128 = 12544 >= 12500), each block padded to a uniform
    tile count T (pad edges have zero weights -> contribute nothing).
  - Per 128-edge tile (one edge per SBUF partition):
      S[e, n]     = (dst_local[e] == n)        one-hot      (DVE is_equal)
      Q[e, (o,i)] = W[e,o,i] * xj[e,i]         products     (DVE mult)
      PSUM[n,(o,i)] += S.T @ Q                 scatter-add  (PE matmul)
    After T tiles: out[n,o] = relu(sum_i PSUM[n,(o,i)] + bias[o])
  - x[src] is gathered on host and streamed densely (adds ~6% traffic vs W).
"""

import os
import sys
import numpy as np

sys.path.insert(0, "/opt/trn_rl_repo")

_LAST_RUN_INFO = {}

N_CORES = 8
BLK = 128          # nodes per destination block (= one-hot window / PSUM rows)
G = 4              # tiles per batched DVE op / W-DMA
IN_C = 16
OUT_C = 16


def _install_ntff_hook():
    """Provide antenv.axon_hooks if the image lacks it (profiling only)."""
    import importlib.util
    import types
    import contextlib
    import ctypes

    if "antenv.axon_hooks" in sys.modules:
        return
    try:
        import antenv.axon_hooks  # noqa: F401
        return
    except ImportError:
        pass

    mod = types.ModuleType("antenv.axon_hooks")
    mod._hook = None
    mod._tried = False

    def set_axon_ntff_profile_hook(h):
        mod._hook = h

    def _via_ctypes(so_path):
        lib = ctypes.CDLL(so_path)
        if not hasattr(lib, "axon_start_nrt_profile"):
            return None
        lib.axon_start_nrt_profile.argtypes = [
            ctypes.POINTER(ctypes.c_int64),
            ctypes.c_size_t,
        ]
        lib.axon_start_nrt_profile.restype = ctypes.c_int64
        lib.axon_stop_nrt_profile.argtypes = [ctypes.c_char_p]
        lib.axon_stop_nrt_profile.restype = ctypes.c_int64

        @contextlib.contextmanager
        def _hook_cm(output_dir, device_ids):
            import jax

            jax.devices()
            if device_ids:
                ids = (ctypes.c_int64 * len(device_ids))(*device_ids)
                rc = lib.axon_start_nrt_profile(ids, len(device_ids))
            else:
                rc = lib.axon_start_nrt_profile(None, 0)
            if rc != 0:
                raise RuntimeError(f"axon_start_nrt_profile rc={rc}")
            try:
                yield
            finally:
                n = lib.axon_stop_nrt_profile(str(output_dir).encode())
                print(f"profile: {n} file(s) written to {output_dir}", file=sys.stderr)

        return _hook_cm

    def get_axon_ntff_profile_hook():
        if mod._hook is None and not mod._tried:
            mod._tried = True
            so = os.environ.get("AXON_PJRT_SO", "/opt/axon/libaxon_pjrt.so")
            if os.path.exists(so):
                try:
                    mod._hook = _via_ctypes(so)
                except OSError:
                    mod._hook = None
        return mod._hook

    mod.set_axon_ntff_profile_hook = set_axon_ntff_profile_hook
    mod.get_axon_ntff_profile_hook = get_axon_ntff_profile_hook
    sys.modules["antenv.axon_hooks"] = mod


def _build_bass(tiles_total, nblk, t_per_blk, stream_fp16):
    import concourse.bass as bass
    import concourse.bacc as bacc
    import concourse.tile as tile
    import concourse.mybir as mybir

    f32 = mybir.dt.float32
    f16 = mybir.dt.bfloat16
    sdt = f16 if stream_fp16 else f32   # dtype of W / xj streams

    nc = bacc.Bacc("TRN2", target_bir_lowering=False, debug=False,
                   num_devices=N_CORES)

    T = t_per_blk
    H = T // 2                       # tiles per local_scatter call (T even)
    NI = H + (H % 2)                 # num_idxs (padded even)
    w_d = nc.dram_tensor("w", [128, tiles_total, 256], sdt, kind="ExternalInput")
    xj_d = nc.dram_tensor("xj", [128, tiles_total, IN_C], sdt, kind="ExternalInput")
    sx_d = nc.dram_tensor("sidx", [128, nblk, 2, NI], mybir.dt.int16,
                          kind="ExternalInput")
    bias_d = nc.dram_tensor("biasb", [128, OUT_C], f32, kind="ExternalInput")
    out_d = nc.dram_tensor("out", [nblk, 128, OUT_C], f32, kind="ExternalOutput")

    with tile.TileContext(nc) as tc:
        with (
            tc.tile_pool(name="wpool", bufs=6) as wpool,
            tc.tile_pool(name="xpool", bufs=4) as xpool,
            tc.tile_pool(name="dpool", bufs=8) as dpool,
            tc.tile_pool(name="spool", bufs=6) as spool,
            tc.tile_pool(name="qpool", bufs=4) as qpool,
            tc.tile_pool(name="opool", bufs=3) as opool,
            tc.tile_pool(name="cpool", bufs=1) as cpool,
            tc.tile_pool(name="psum", bufs=4, space="PSUM") as psum_pool,
        ):
            bias_t = cpool.tile([128, OUT_C], f32, tag="bias")
            nc.sync.dma_start(bias_t[:], bias_d[:])
            ones_t = cpool.tile([128, NI], f16, tag="ones")
            nc.vector.memset(ones_t[:], 1.0)

            ps_tiles = {}

            def emit_front(b):
                ps = psum_pool.tile([128, 256], f32)
                ps_tiles[b] = ps
                base = b * T
                # one DMA per stream per block (sequencer-issue amortized)
                wt = wpool.tile([128, T, 256], sdt, tag="wt")
                half = T // 2
                nc.sync.dma_start(wt[:, :half, :], w_d[:, base:base + half, :])
                nc.sync.dma_start(wt[:, half:, :],
                                  w_d[:, base + half:base + T, :])
                xt = xpool.tile([128, T, IN_C], sdt, tag="xt")
                nc.scalar.dma_start(xt[:], xj_d[:, base:base + T, :])
                sx = dpool.tile([128, 2, NI], mybir.dt.int16, tag="sx")
                nc.scalar.dma_start(sx[:], sx_d[:, b, :, :])

                # one-hot S built by GpSimd local scatter (zero + ones)
                st = spool.tile([128, T, BLK], f16, tag="st")
                for h in range(2):
                    nc.gpsimd.local_scatter(
                        st[:, h * H:(h + 1) * H, :].rearrange(
                            "p t n -> p (t n)"),
                        ones_t[:],
                        sx[:, h, :],
                        channels=128,
                        num_elems=H * BLK,
                        num_idxs=NI,
                    )
                qt = qpool.tile([128, T, OUT_C, IN_C], f16, tag="qt")
                nc.vector.tensor_tensor(
                    qt[:],
                    wt[:].rearrange("p g (o i) -> p g o i", i=IN_C),
                    xt[:].unsqueeze(2).broadcast_to([128, T, OUT_C, IN_C]),
                    op=mybir.AluOpType.mult,
                )
                for k in range(T):
                    nc.tensor.matmul(
                        ps[:],
                        st[:, k, :],
                        qt[:, k, :, :],
                        start=(k == 0),
                        stop=(k == T - 1),
                    )

            def emit_back(b):
                ps = ps_tiles.pop(b)
                ot = opool.tile([128, OUT_C], f32, tag="ot")
                nc.vector.tensor_reduce(
                    ot[:],
                    ps[:].rearrange("p (o i) -> p o i", i=IN_C),
                    axis=mybir.AxisListType.X,
                    op=mybir.AluOpType.add,
                )
                ob = opool.tile([128, OUT_C], f32, tag="ob")
                nc.vector.tensor_tensor(
                    ob[:], ot[:], bias_t[:], op=mybir.AluOpType.add)
                orl = opool.tile([128, OUT_C], f32, tag="orl")
                nc.vector.tensor_relu(orl[:], ob[:])
                nc.scalar.dma_start(out_d[b], orl[:])

            # software pipeline: block b's PSUM eviction is emitted after
            # block b+1's front phase, so the DVE never stalls on the PE
            # finishing the current block (in-order engine queues).
            for b in range(nblk):
                emit_front(b)
                if b >= 2:
                    emit_back(b - 2)
            emit_back(nblk - 2)
            emit_back(nblk - 1)

    nc.compile()
    return nc


def kernel(x, edge_index, edge_attr, weights_matrices, bias,
           input_size, output_size, **_unused):
    _install_ntff_hook()
    import ml_dtypes

    stream_fp16 = bool(int(os.environ.get("GNN_STREAM_FP16", "1")))
    import ml_dtypes
    sdt_np = ml_dtypes.bfloat16 if stream_fp16 else np.float32

    x = np.asarray(x, dtype=np.float32)
    edge_index = np.asarray(edge_index)
    W = np.asarray(weights_matrices, dtype=np.float32)
    bias = np.asarray(bias, dtype=np.float32)

    N = x.shape[0]
    E = edge_index.shape[1]
    n_per_core = (N + N_CORES - 1) // N_CORES          # 12500
    nblk = (n_per_core + BLK - 1) // BLK               # 98

    src = edge_index[0].astype(np.int64)
    dst = edge_index[1].astype(np.int64)

    core = dst // n_per_core
    local = dst - core * n_per_core
    blk = local // BLK
    dstl = (local - blk * BLK).astype(np.float32)       # in [0,128)

    # group edges by (core, block)
    key = core * nblk + blk
    order = np.argsort(key, kind="stable")
    key_sorted = key[order]
    counts = np.bincount(key_sorted, minlength=N_CORES * nblk)
    t_per_blk = int((counts.max() + BLK - 1) // BLK)    # uniform tiles/block
    t_per_blk += t_per_blk % 2                          # even (scatter halves)
    tiles_total = nblk * t_per_blk
    epc = tiles_total * BLK                             # padded edges per core

    # slot position of each sorted edge inside its (core, block) bucket
    group_start = np.zeros(N_CORES * nblk + 1, dtype=np.int64)
    np.cumsum(counts, out=group_start[1:])
    within = np.arange(E, dtype=np.int64) - group_start[key_sorted]
    core_s = key_sorted // nblk
    blk_s = key_sorted - core_s * nblk
    pos = blk_s * (t_per_blk * BLK) + within            # slot within core

    # perm[c, slot] = original edge id, -1 for padding
    perm = np.full((N_CORES, epc), -1, dtype=np.int64)
    perm[core_s, pos] = order

    pad_mask = perm < 0
    perm_c = np.where(pad_mask, 0, perm)

    # build per-core streams; layout [cores, 128 partitions, tiles, ...]
    # edge slot s -> tile s // 128, partition s % 128
    def to_tiles(a):
        # a: [N_CORES, epc, F] -> [N_CORES, 128, tiles_total, F]
        F = a.shape[-1]
        return np.ascontiguousarray(
            a.reshape(N_CORES, tiles_total, BLK, F).transpose(0, 2, 1, 3))

    Wf = W.reshape(E, 256)
    w_perm = Wf[perm_c].astype(sdt_np)
    w_perm[pad_mask] = 0.0
    w_perm = to_tiles(w_perm)

    xj = x[src[perm_c]].astype(sdt_np)
    xj = to_tiles(xj)

    # scatter indices for the one-hot build:
    # edge slot s -> partition s%128, tile s//128; tile -> block b = tile//T,
    # k = tile%T, half h = k//H, idx value = (k%H)*BLK + dst_local  (-1 = pad)
    T = t_per_blk
    H = T // 2
    NI = H + (H % 2)
    s_arr = np.arange(epc, dtype=np.int64)
    p_arr = s_arr % BLK
    tile_arr = s_arr // BLK
    b_arr = tile_arr // T
    k_arr = tile_arr % T
    h_arr = k_arr // H
    kih_arr = k_arr % H
    dl_perm = dstl[perm_c].astype(np.int64)             # [N_CORES, epc]
    val = (kih_arr[None, :] * BLK + dl_perm).astype(np.int16)
    val[pad_mask] = -1
    sidx = np.full((N_CORES, 128, nblk, 2, NI), -1, dtype=np.int16)
    c_idx = np.repeat(np.arange(N_CORES), epc)
    sidx[c_idx, np.tile(p_arr, N_CORES), np.tile(b_arr, N_CORES),
         np.tile(h_arr, N_CORES), np.tile(kih_arr, N_CORES)] = val.ravel()

    bias_b = np.broadcast_to(bias, (128, OUT_C)).astype(np.float32)

    from concourse.bass_utils import run_bass_kernel_spmd

    nc = _build_bass(tiles_total, nblk, t_per_blk, stream_fp16)

    in_maps = [
        {
            "w": np.ascontiguousarray(w_perm[c]),
            "xj": np.ascontiguousarray(xj[c]),
            "sidx": np.ascontiguousarray(sidx[c]),
            "biasb": bias_b,
        }
        for c in range(N_CORES)
    ]

    trace = bool(int(os.environ.get("GNN_TRACE", "0")))
    res = run_bass_kernel_spmd(
        nc, in_maps, core_ids=list(range(N_CORES)), trace=trace)

    _LAST_RUN_INFO.clear()
    _LAST_RUN_INFO.update(
        exec_time_ns=res.exec_time_ns,
        mean_exec_time_ns=res.mean_exec_time_ns,
        tiles_total=tiles_total,
        t_per_blk=t_per_blk,
        profile_json=res.profile_json,
        instructions_and_trace=res.instructions_and_trace,
    )

    out = np.concatenate(
        [res.results[c]["out"].reshape(nblk * BLK, OUT_C)[:n_per_core]
         for c in range(N_CORES)], axis=0)
    return out[:N]


# revision 27
# speedup vs baseline: 1.2510x; 1.0856x over previous
"""Trainium2 Bass kernel for CustomGraphConv message passing.

reference computation:
    msg  = einsum('eoi,ei->eo', W, x[src])          # per-edge matvec
    aggr = segment_sum(msg, dst, num_segments=N)     # scatter-add
    out  = relu(aggr + bias)

Strategy (8 NeuronCores):
  - Partition OUTPUT nodes across cores: core c owns dst in [c*N/8, (c+1)*N/8).
    Edges are routed to the core owning their destination -> no all-reduce.
  - Within a core, edges are binned by destination block of 128 nodes.
    Blocks are processed as "windows" sorted by descending edge count; each
    sorted position is padded only to the max tile count across the 8 cores
    (SPMD needs one instruction stream), minimizing zero-weight padding.
  - Per 128-edge tile (one edge per SBUF partition):
      S[e, n]     = one-hot of dst_local[e]      (GpSimd local_scatter)
      Q[e, (o,i)] = W[e,o,i] * xj[e,i]           (DVE mult, bf16 out)
      PSUM[n,(o,i)] += S.T @ Q                   (PE matmul, f32 accum)
    After T_w tiles: out[n,o] = relu(sum_i PSUM[n,(o,i)] + bias[o])
  - x[src] is gathered on host and streamed densely (adds ~6% traffic vs W).
  - Queues are single-purpose (sync=W, scalar=xj/sidx/out, Pool=scatter,
    DVE=mult/reduce/bias/relu) and PSUM eviction is software-pipelined two
    windows behind, so no in-order sequencer ever head-of-line blocks a
    prefetch DMA on a drain semaphore.
"""

import os
import sys
import numpy as np

sys.path.insert(0, "/opt/trn_rl_repo")

_LAST_RUN_INFO = {}

N_CORES = 8
BLK = 128          # nodes per destination block (= one-hot window / PSUM rows)
IN_C = 16
OUT_C = 16


def _install_ntff_hook():
    """Provide antenv.axon_hooks if the image lacks it (profiling only)."""
    import types
    import contextlib
    import ctypes

    if "antenv.axon_hooks" in sys.modules:
        return
    try:
        import antenv.axon_hooks  # noqa: F401
        return
    except ImportError:
        pass

    mod = types.ModuleType("antenv.axon_hooks")
    mod._hook = None
    mod._tried = False

    def set_axon_ntff_profile_hook(h):
        mod._hook = h

    def _via_ctypes(so_path):
        lib = ctypes.CDLL(so_path)
        if not hasattr(lib, "axon_start_nrt_profile"):
            return None
        lib.axon_start_nrt_profile.argtypes = [
            ctypes.POINTER(ctypes.c_int64),
            ctypes.c_size_t,
        ]
        lib.axon_start_nrt_profile.restype = ctypes.c_int64
        lib.axon_stop_nrt_profile.argtypes = [ctypes.c_char_p]
        lib.axon_stop_nrt_profile.restype = ctypes.c_int64

        @contextlib.contextmanager
        def _hook_cm(output_dir, device_ids):
            import jax

            jax.devices()
            if device_ids:
                ids = (ctypes.c_int64 * len(device_ids))(*device_ids)
                rc = lib.axon_start_nrt_profile(ids, len(device_ids))
            else:
                rc = lib.axon_start_nrt_profile(None, 0)
            if rc != 0:
                raise RuntimeError(f"axon_start_nrt_profile rc={rc}")
            try:
                yield
            finally:
                n = lib.axon_stop_nrt_profile(str(output_dir).encode())
                print(f"profile: {n} file(s) written to {output_dir}",
                      file=sys.stderr)

        return _hook_cm

    def get_axon_ntff_profile_hook():
        if mod._hook is None and not mod._tried:
            mod._tried = True
            so = os.environ.get("AXON_PJRT_SO", "/opt/axon/libaxon_pjrt.so")
            if os.path.exists(so):
                try:
                    mod._hook = _via_ctypes(so)
                except OSError:
                    mod._hook = None
        return mod._hook

    mod.set_axon_ntff_profile_hook = set_axon_ntff_profile_hook
    mod.get_axon_ntff_profile_hook = get_axon_ntff_profile_hook
    sys.modules["antenv.axon_hooks"] = mod


def _build_bass(tiles_total, env, sidx_tot, stream_fp16):
    import concourse.bacc as bacc
    import concourse.tile as tile
    import concourse.mybir as mybir

    f32 = mybir.dt.float32
    f16 = mybir.dt.bfloat16
    sdt = f16 if stream_fp16 else f32   # dtype of W / xj streams
    nblk = len(env)
    t_max = max(env)
    off = [0]
    for t in env:
        off.append(off[-1] + t)
    so = [0]
    for t in env:
        h = t // 2
        so.append(so[-1] + 2 * (h + h % 2))

    nc = bacc.Bacc("TRN2", target_bir_lowering=False, debug=False,
                   num_devices=N_CORES)

    w_d = nc.dram_tensor("w", [128, tiles_total, 256], sdt,
                         kind="ExternalInput")
    xj_d = nc.dram_tensor("xj", [128, tiles_total, IN_C], sdt,
                          kind="ExternalInput")
    sx_d = nc.dram_tensor("sidx", [128, sidx_tot], mybir.dt.int16,
                          kind="ExternalInput")
    bias_d = nc.dram_tensor("biasb", [128, OUT_C], f32, kind="ExternalInput")
    out_d = nc.dram_tensor("out", [nblk, 128, OUT_C], f32,
                           kind="ExternalOutput")

    with tile.TileContext(nc) as tc:
        with (
            tc.tile_pool(name="wpool", bufs=6) as wpool,
            tc.tile_pool(name="xpool", bufs=6) as xpool,
            tc.tile_pool(name="dpool", bufs=8) as dpool,
            tc.tile_pool(name="spool", bufs=6) as spool,
            tc.tile_pool(name="qpool", bufs=4) as qpool,
            tc.tile_pool(name="opool", bufs=4) as opool,
            tc.tile_pool(name="cpool", bufs=1) as cpool,
            tc.tile_pool(name="psum", bufs=4, space="PSUM") as psum_pool,
        ):
            bias_t = cpool.tile([128, OUT_C], f32, tag="bias")
            nc.sync.dma_start(bias_t[:], bias_d[:])
            ones_t = cpool.tile([128, t_max], f16, tag="ones")
            nc.vector.memset(ones_t[:], 1.0)

            ps_tiles = {}

            def emit_front(b):
                T = env[b]
                H = T // 2
                NI = H + (H % 2)
                base = off[b]
                ps = psum_pool.tile([128, 256], f32)
                ps_tiles[b] = ps
                wt = wpool.tile([128, t_max, 256], sdt, tag="wt")
                nc.sync.dma_start(wt[:, :H, :], w_d[:, base:base + H, :])
                nc.sync.dma_start(wt[:, H:T, :], w_d[:, base + H:base + T, :])
                xt = xpool.tile([128, t_max, IN_C], sdt, tag="xt")
                nc.scalar.dma_start(xt[:, :T, :], xj_d[:, base:base + T, :])
                sx = dpool.tile([128, 2 * (t_max // 2 + 1)], mybir.dt.int16,
                                tag="sx")
                nc.scalar.dma_start(sx[:, :2 * NI], sx_d[:, so[b]:so[b + 1]])

                st = spool.tile([128, t_max, BLK], f16, tag="st")
                qt = qpool.tile([128, t_max, OUT_C, IN_C], f16, tag="qt")
                for h in range(2):
                    lo, hi = h * H, (h + 1) * H
                    # one-hot S for this half (GpSimd: zero + scatter ones)
                    nc.gpsimd.local_scatter(
                        st[:, lo:hi, :].rearrange("p t n -> p (t n)"),
                        ones_t[:, :NI],
                        sx[:, h * NI:(h + 1) * NI],
                        channels=128,
                        num_elems=H * BLK,
                        num_idxs=NI,
                    )
                    # per-edge products for this half (starts on half-DMA)
                    nc.vector.tensor_tensor(
                        qt[:, lo:hi, :, :],
                        wt[:, lo:hi, :].rearrange("p g (o i) -> p g o i",
                                                  i=IN_C),
                        xt[:, lo:hi, :].unsqueeze(2).broadcast_to(
                            [128, H, OUT_C, IN_C]),
                        op=mybir.AluOpType.mult,
                    )
                for k in range(T):
                    nc.tensor.matmul(
                        ps[:],
                        st[:, k, :],
                        qt[:, k, :, :],
                        start=(k == 0),
                        stop=(k == T - 1),
                    )

            def emit_back(b):
                ps = ps_tiles.pop(b)
                ot = opool.tile([128, OUT_C], f32, tag="ot")
                nc.vector.tensor_reduce(
                    ot[:],
                    ps[:].rearrange("p (o i) -> p o i", i=IN_C),
                    axis=mybir.AxisListType.X,
                    op=mybir.AluOpType.add,
                )
                ob = opool.tile([128, OUT_C], f32, tag="ob")
                nc.vector.tensor_tensor(
                    ob[:], ot[:], bias_t[:], op=mybir.AluOpType.add)
                orl = opool.tile([128, OUT_C], f32, tag="orl")
                nc.vector.tensor_relu(orl[:], ob[:])
                nc.scalar.dma_start(out_d[b], orl[:])

            # software pipeline: window b's PSUM eviction is emitted two
            # windows later so in-order engine queues never stall on the PE
            # finishing the current window.
            for b in range(nblk):
                emit_front(b)
                if b >= 2:
                    emit_back(b - 2)
            emit_back(nblk - 2)
            emit_back(nblk - 1)

    nc.compile()
    return nc


def kernel(x, edge_index, edge_attr, weights_matrices, bias,
           input_size, output_size, **_unused):
    _install_ntff_hook()
    import ml_dtypes

    stream_fp16 = bool(int(os.environ.get("GNN_STREAM_FP16", "1")))
    sdt_np = ml_dtypes.bfloat16 if stream_fp16 else np.float32

    x = np.asarray(x, dtype=np.float32)
    edge_index = np.asarray(edge_index)
    W = np.asarray(weights_matrices, dtype=np.float32)
    bias = np.asarray(bias, dtype=np.float32)

    N = x.shape[0]
    E = edge_index.shape[1]
    n_per_core = (N + N_CORES - 1) // N_CORES          # 12500
    nblk = (n_per_core + BLK - 1) // BLK               # 98

    src = edge_index[0].astype(np.int64)
    dst = edge_index[1].astype(np.int64)

    core = dst // n_per_core
    local = dst - core * n_per_core
    blk = local // BLK
    dstl = (local - blk * BLK).astype(np.int64)         # in [0,128)

    # group edges by (core, block)
    key = core * nblk + blk
    order = np.argsort(key, kind="stable")
    key_sorted = key[order]
    counts = np.bincount(key_sorted, minlength=N_CORES * nblk)
    t_cb = (counts.reshape(N_CORES, nblk) + BLK - 1) // BLK
    t_cb = np.maximum(t_cb, 1)

    # windows: per core, blocks sorted by descending tile count; pad each
    # sorted position to the max across cores (one SPMD instruction stream)
    order_c = np.argsort(-t_cb, axis=1, kind="stable")  # [cores, nblk]
    t_sorted = np.take_along_axis(t_cb, order_c, axis=1)
    env = t_sorted.max(axis=0).astype(np.int64)
    env += env % 2                                      # even (scatter halves)
    off = np.zeros(nblk + 1, np.int64)
    np.cumsum(env, out=off[1:])
    tiles_total = int(off[-1])
    epc = tiles_total * BLK                             # padded edges per core
    H_w = env // 2
    NI_w = H_w + (H_w % 2)
    so = np.zeros(nblk + 1, np.int64)
    np.cumsum(2 * NI_w, out=so[1:])
    sidx_tot = int(so[-1])
    win_cb = np.empty_like(order_c)
    win_cb[np.arange(N_CORES)[:, None], order_c] = np.arange(nblk)[None, :]

    # slot position of each sorted edge inside its (core, window) bucket
    group_start = np.zeros(N_CORES * nblk + 1, dtype=np.int64)
    np.cumsum(counts, out=group_start[1:])
    within = np.arange(E, dtype=np.int64) - group_start[key_sorted]
    core_s = key_sorted // nblk
    blk_s = key_sorted - core_s * nblk
    win_s = win_cb[core_s, blk_s]
    pos = off[win_s] * BLK + within                     # slot within core

    # perm[c, slot] = original edge id, -1 for padding
    perm = np.full((N_CORES, epc), -1, dtype=np.int64)
    perm[core_s, pos] = order

    pad_mask = perm < 0
    perm_c = np.where(pad_mask, 0, perm)

    # per-core streams; layout [cores, 128 partitions, tiles, ...]
    # edge slot s -> tile s // 128, partition s % 128
    def to_tiles(a):
        F = a.shape[-1]
        return np.ascontiguousarray(
            a.reshape(N_CORES, tiles_total, BLK, F).transpose(0, 2, 1, 3))

    Wf = W.reshape(E, IN_C * OUT_C)
    w_perm = Wf[perm_c].astype(sdt_np)
    w_perm[pad_mask] = 0.0
    w_perm = to_tiles(w_perm)

    xj = x[src[perm_c]].astype(sdt_np)
    xj = to_tiles(xj)

    # scatter indices for the one-hot build: edge slot s -> partition s%128,
    # tile s//128 -> window w, tile-in-window k, half h = k//H_w,
    # sidx column so[w] + h*NI_w + (k%H_w), value (k%H_w)*BLK + dst_local
    s_arr = np.arange(epc, dtype=np.int64)
    p_arr = s_arr % BLK
    tile_arr = s_arr // BLK
    w_arr = np.searchsorted(off, tile_arr, side="right") - 1
    k_arr = tile_arr - off[w_arr]
    h_arr = k_arr // H_w[w_arr]
    kih_arr = k_arr - h_arr * H_w[w_arr]
    col_arr = so[w_arr] + h_arr * NI_w[w_arr] + kih_arr
    dl_perm = dstl[perm_c]                              # [cores, epc]
    val = (kih_arr[None, :] * BLK + dl_perm).astype(np.int16)
    val[pad_mask] = -1
    sidx = np.full((N_CORES, 128, sidx_tot), -1, dtype=np.int16)
    c_idx = np.repeat(np.arange(N_CORES), epc)
    sidx[c_idx, np.tile(p_arr, N_CORES), np.tile(col_arr, N_CORES)] = \
        val.ravel()

    bias_b = np.broadcast_to(bias, (128, OUT_C)).astype(np.float32)

    from concourse.bass_utils import run_bass_kernel_spmd

    nc = _build_bass(tiles_total, [int(t) for t in env], sidx_tot,
                     stream_fp16)

    in_maps = [
        {
            "w": np.ascontiguousarray(w_perm[c]),
            "xj": np.ascontiguousarray(xj[c]),
            "sidx": np.ascontiguousarray(sidx[c]),
            "biasb": bias_b,
        }
        for c in range(N_CORES)
    ]

    trace = bool(int(os.environ.get("GNN_TRACE", "0")))
    res = run_bass_kernel_spmd(
        nc, in_maps, core_ids=list(range(N_CORES)), trace=trace)

    _LAST_RUN_INFO.clear()
    _LAST_RUN_INFO.update(
        exec_time_ns=res.exec_time_ns,
        mean_exec_time_ns=res.mean_exec_time_ns,
        tiles_total=tiles_total,
        t_per_blk=float(np.mean(env)),
        profile_json=res.profile_json,
        instructions_and_trace=res.instructions_and_trace,
    )

    # un-permute windows -> blocks, concatenate cores
    outs = []
    for c in range(N_CORES):
        by_win = res.results[c]["out"]                  # [nblk, 128, OUT_C]
        by_blk = np.empty_like(by_win)
        by_blk[order_c[c]] = by_win
        outs.append(by_blk.reshape(nblk * BLK, OUT_C)[:n_per_core])
    out = np.concatenate(outs, axis=0)
    return out[:N]


# revision 28
# speedup vs baseline: 1.2662x; 1.0122x over previous
"""Trainium2 Bass kernel for CustomGraphConv message passing.

reference computation:
    msg  = einsum('eoi,ei->eo', W, x[src])          # per-edge matvec
    aggr = segment_sum(msg, dst, num_segments=N)     # scatter-add
    out  = relu(aggr + bias)

Strategy (8 NeuronCores):
  - Partition OUTPUT nodes across cores: core c owns dst in [c*N/8, (c+1)*N/8).
    Edges are routed to the core owning their destination -> no all-reduce.
  - Within a core, edges are binned by destination block of 128 nodes.
    Blocks are processed as "windows" sorted by descending edge count; each
    sorted position is padded only to the max tile count across the 8 cores
    (SPMD needs one instruction stream), minimizing zero-weight padding.
  - Per 128-edge tile (one edge per SBUF partition):
      S[e, n]     = one-hot of dst_local[e]      (GpSimd local_scatter)
      Q[e, (o,i)] = W[e,o,i] * xj[e,i]           (DVE mult, bf16 out)
      PSUM[n,(o,i)] += S.T @ Q                   (PE matmul, f32 accum)
    After T_w tiles: out[n,o] = relu(sum_i PSUM[n,(o,i)] + bias[o])
  - x[src] is gathered on host and streamed densely (adds ~6% traffic vs W).
  - Queues are single-purpose (sync=W, scalar=xj/sidx/out, Pool=scatter,
    DVE=mult/reduce/bias/relu) and PSUM eviction is software-pipelined two
    windows behind, so no in-order sequencer ever head-of-line blocks a
    prefetch DMA on a drain semaphore.
"""

import os
import sys
import numpy as np

sys.path.insert(0, "/opt/trn_rl_repo")

_LAST_RUN_INFO = {}

N_CORES = 8
BLK = 128          # nodes per destination block (= one-hot window / PSUM rows)
IN_C = 16
OUT_C = 16


def _install_ntff_hook():
    """Provide antenv.axon_hooks if the image lacks it (profiling only)."""
    import types
    import contextlib
    import ctypes

    if "antenv.axon_hooks" in sys.modules:
        return
    try:
        import antenv.axon_hooks  # noqa: F401
        return
    except ImportError:
        pass

    mod = types.ModuleType("antenv.axon_hooks")
    mod._hook = None
    mod._tried = False

    def set_axon_ntff_profile_hook(h):
        mod._hook = h

    def _via_ctypes(so_path):
        lib = ctypes.CDLL(so_path)
        if not hasattr(lib, "axon_start_nrt_profile"):
            return None
        lib.axon_start_nrt_profile.argtypes = [
            ctypes.POINTER(ctypes.c_int64),
            ctypes.c_size_t,
        ]
        lib.axon_start_nrt_profile.restype = ctypes.c_int64
        lib.axon_stop_nrt_profile.argtypes = [ctypes.c_char_p]
        lib.axon_stop_nrt_profile.restype = ctypes.c_int64

        @contextlib.contextmanager
        def _hook_cm(output_dir, device_ids):
            import jax

            jax.devices()
            if device_ids:
                ids = (ctypes.c_int64 * len(device_ids))(*device_ids)
                rc = lib.axon_start_nrt_profile(ids, len(device_ids))
            else:
                rc = lib.axon_start_nrt_profile(None, 0)
            if rc != 0:
                raise RuntimeError(f"axon_start_nrt_profile rc={rc}")
            try:
                yield
            finally:
                n = lib.axon_stop_nrt_profile(str(output_dir).encode())
                print(f"profile: {n} file(s) written to {output_dir}",
                      file=sys.stderr)

        return _hook_cm

    def get_axon_ntff_profile_hook():
        if mod._hook is None and not mod._tried:
            mod._tried = True
            so = os.environ.get("AXON_PJRT_SO", "/opt/axon/libaxon_pjrt.so")
            if os.path.exists(so):
                try:
                    mod._hook = _via_ctypes(so)
                except OSError:
                    mod._hook = None
        return mod._hook

    mod.set_axon_ntff_profile_hook = set_axon_ntff_profile_hook
    mod.get_axon_ntff_profile_hook = get_axon_ntff_profile_hook
    sys.modules["antenv.axon_hooks"] = mod


def _build_bass(tiles_total, env, sidx_tot, stream_fp16):
    import concourse.bacc as bacc
    import concourse.tile as tile
    import concourse.mybir as mybir

    f32 = mybir.dt.float32
    f16 = mybir.dt.bfloat16
    sdt = f16 if stream_fp16 else f32   # dtype of W / xj streams
    nblk = len(env)
    t_max = max(env)
    off = [0]
    for t in env:
        off.append(off[-1] + t)
    so = [0]
    for t in env:
        h = t // 2
        so.append(so[-1] + 2 * (h + h % 2))

    nc = bacc.Bacc("TRN2", target_bir_lowering=False, debug=False,
                   num_devices=N_CORES)

    w_d = nc.dram_tensor("w", [128, tiles_total, 256], sdt,
                         kind="ExternalInput")
    xj_d = nc.dram_tensor("xj", [128, tiles_total, IN_C], sdt,
                          kind="ExternalInput")
    sx_d = nc.dram_tensor("sidx", [128, sidx_tot], mybir.dt.int16,
                          kind="ExternalInput")
    bias_d = nc.dram_tensor("biasb", [128, OUT_C], f32, kind="ExternalInput")
    out_d = nc.dram_tensor("out", [nblk, 128, OUT_C], f32,
                           kind="ExternalOutput")

    with tile.TileContext(nc) as tc:
        with (
            tc.tile_pool(name="wpool", bufs=9) as wpool,
            tc.tile_pool(name="xpool", bufs=6) as xpool,
            tc.tile_pool(name="dpool", bufs=8) as dpool,
            tc.tile_pool(name="spool", bufs=4) as spool,
            tc.tile_pool(name="qpool", bufs=4) as qpool,
            tc.tile_pool(name="opool", bufs=4) as opool,
            tc.tile_pool(name="cpool", bufs=1) as cpool,
            tc.tile_pool(name="psum", bufs=4, space="PSUM") as psum_pool,
        ):
            bias_t = cpool.tile([128, OUT_C], f32, tag="bias")
            nc.sync.dma_start(bias_t[:], bias_d[:])
            ones_t = cpool.tile([128, t_max], f16, tag="ones")
            nc.vector.memset(ones_t[:], 1.0)

            ps_tiles = {}

            def emit_front(b):
                T = env[b]
                H = T // 2
                NI = H + (H % 2)
                base = off[b]
                ps = psum_pool.tile([128, 256], f32)
                ps_tiles[b] = ps
                wt = wpool.tile([128, t_max, 256], sdt, tag="wt")
                qb = sorted({0, H // 2, H, H + (T - H) // 2, T})
                for lo, hi in zip(qb[:-1], qb[1:]):
                    nc.sync.dma_start(wt[:, lo:hi, :],
                                      w_d[:, base + lo:base + hi, :])
                xt = xpool.tile([128, t_max, IN_C], sdt, tag="xt")
                nc.scalar.dma_start(xt[:, :T, :], xj_d[:, base:base + T, :])
                sx = dpool.tile([128, 2 * (t_max // 2 + 1)], mybir.dt.int16,
                                tag="sx")
                nc.scalar.dma_start(sx[:, :2 * NI], sx_d[:, so[b]:so[b + 1]])

                st = spool.tile([128, t_max, BLK], f16, tag="st")
                qt = qpool.tile([128, t_max, OUT_C, IN_C], f16, tag="qt")
                for h in range(2):
                    lo, hi = h * H, (h + 1) * H
                    # one-hot S for this half (GpSimd: zero + scatter ones)
                    nc.gpsimd.local_scatter(
                        st[:, lo:hi, :].rearrange("p t n -> p (t n)"),
                        ones_t[:, :NI],
                        sx[:, h * NI:(h + 1) * NI],
                        channels=128,
                        num_elems=H * BLK,
                        num_idxs=NI,
                    )
                for lo, hi in zip(qb[:-1], qb[1:]):
                    # per-edge products per W-DMA chunk (early start)
                    nc.vector.tensor_tensor(
                        qt[:, lo:hi, :, :],
                        wt[:, lo:hi, :].rearrange("p g (o i) -> p g o i",
                                                  i=IN_C),
                        xt[:, lo:hi, :].unsqueeze(2).broadcast_to(
                            [128, hi - lo, OUT_C, IN_C]),
                        op=mybir.AluOpType.mult,
                    )
                for k in range(T):
                    nc.tensor.matmul(
                        ps[:],
                        st[:, k, :],
                        qt[:, k, :, :],
                        start=(k == 0),
                        stop=(k == T - 1),
                    )

            def emit_back(b):
                ps = ps_tiles.pop(b)
                ot = opool.tile([128, OUT_C], f32, tag="ot")
                nc.vector.tensor_reduce(
                    ot[:],
                    ps[:].rearrange("p (o i) -> p o i", i=IN_C),
                    axis=mybir.AxisListType.X,
                    op=mybir.AluOpType.add,
                )
                ob = opool.tile([128, OUT_C], f32, tag="ob")
                nc.vector.tensor_tensor(
                    ob[:], ot[:], bias_t[:], op=mybir.AluOpType.add)
                orl = opool.tile([128, OUT_C], f32, tag="orl")
                nc.vector.tensor_relu(orl[:], ob[:])
                nc.scalar.dma_start(out_d[b], orl[:])

            # software pipeline: window b's PSUM eviction is emitted two
            # windows later so in-order engine queues never stall on the PE
            # finishing the current window.
            for b in range(nblk):
                emit_front(b)
                if b >= 2:
                    emit_back(b - 2)
            emit_back(nblk - 2)
            emit_back(nblk - 1)

    nc.compile()
    return nc


def kernel(x, edge_index, edge_attr, weights_matrices, bias,
           input_size, output_size, **_unused):
    _install_ntff_hook()
    import ml_dtypes

    stream_fp16 = bool(int(os.environ.get("GNN_STREAM_FP16", "1")))
    sdt_np = ml_dtypes.bfloat16 if stream_fp16 else np.float32

    x = np.asarray(x, dtype=np.float32)
    edge_index = np.asarray(edge_index)
    W = np.asarray(weights_matrices, dtype=np.float32)
    bias = np.asarray(bias, dtype=np.float32)

    N = x.shape[0]
    E = edge_index.shape[1]
    n_per_core = (N + N_CORES - 1) // N_CORES          # 12500
    nblk = (n_per_core + BLK - 1) // BLK               # 98

    src = edge_index[0].astype(np.int64)
    dst = edge_index[1].astype(np.int64)

    core = dst // n_per_core
    local = dst - core * n_per_core
    blk = local // BLK
    dstl = (local - blk * BLK).astype(np.int64)         # in [0,128)

    # group edges by (core, block)
    key = core * nblk + blk
    order = np.argsort(key, kind="stable")
    key_sorted = key[order]
    counts = np.bincount(key_sorted, minlength=N_CORES * nblk)
    t_cb = (counts.reshape(N_CORES, nblk) + BLK - 1) // BLK
    t_cb = np.maximum(t_cb, 1)

    # windows: per core, blocks sorted by descending tile count; pad each
    # sorted position to the max across cores (one SPMD instruction stream)
    order_c = np.argsort(-t_cb, axis=1, kind="stable")  # [cores, nblk]
    t_sorted = np.take_along_axis(t_cb, order_c, axis=1)
    env = t_sorted.max(axis=0).astype(np.int64)
    env += env % 2                                      # even (scatter halves)
    off = np.zeros(nblk + 1, np.int64)
    np.cumsum(env, out=off[1:])
    tiles_total = int(off[-1])
    epc = tiles_total * BLK                             # padded edges per core
    H_w = env // 2
    NI_w = H_w + (H_w % 2)
    so = np.zeros(nblk + 1, np.int64)
    np.cumsum(2 * NI_w, out=so[1:])
    sidx_tot = int(so[-1])
    win_cb = np.empty_like(order_c)
    win_cb[np.arange(N_CORES)[:, None], order_c] = np.arange(nblk)[None, :]

    # slot position of each sorted edge inside its (core, window) bucket
    group_start = np.zeros(N_CORES * nblk + 1, dtype=np.int64)
    np.cumsum(counts, out=group_start[1:])
    within = np.arange(E, dtype=np.int64) - group_start[key_sorted]
    core_s = key_sorted // nblk
    blk_s = key_sorted - core_s * nblk
    win_s = win_cb[core_s, blk_s]
    pos = off[win_s] * BLK + within                     # slot within core

    # perm[c, slot] = original edge id, -1 for padding
    perm = np.full((N_CORES, epc), -1, dtype=np.int64)
    perm[core_s, pos] = order

    pad_mask = perm < 0
    perm_c = np.where(pad_mask, 0, perm)

    # per-core streams; layout [cores, 128 partitions, tiles, ...]
    # edge slot s -> tile s // 128, partition s % 128
    def to_tiles(a):
        F = a.shape[-1]
        return np.ascontiguousarray(
            a.reshape(N_CORES, tiles_total, BLK, F).transpose(0, 2, 1, 3))

    Wf = W.reshape(E, IN_C * OUT_C)
    w_perm = Wf[perm_c].astype(sdt_np)
    w_perm[pad_mask] = 0.0
    w_perm = to_tiles(w_perm)

    xj = x[src[perm_c]].astype(sdt_np)
    xj = to_tiles(xj)

    # scatter indices for the one-hot build: edge slot s -> partition s%128,
    # tile s//128 -> window w, tile-in-window k, half h = k//H_w,
    # sidx column so[w] + h*NI_w + (k%H_w), value (k%H_w)*BLK + dst_local
    s_arr = np.arange(epc, dtype=np.int64)
    p_arr = s_arr % BLK
    tile_arr = s_arr // BLK
    w_arr = np.searchsorted(off, tile_arr, side="right") - 1
    k_arr = tile_arr - off[w_arr]
    h_arr = k_arr // H_w[w_arr]
    kih_arr = k_arr - h_arr * H_w[w_arr]
    col_arr = so[w_arr] + h_arr * NI_w[w_arr] + kih_arr
    dl_perm = dstl[perm_c]                              # [cores, epc]
    val = (kih_arr[None, :] * BLK + dl_perm).astype(np.int16)
    val[pad_mask] = -1
    sidx = np.full((N_CORES, 128, sidx_tot), -1, dtype=np.int16)
    c_idx = np.repeat(np.arange(N_CORES), epc)
    sidx[c_idx, np.tile(p_arr, N_CORES), np.tile(col_arr, N_CORES)] = \
        val.ravel()

    bias_b = np.broadcast_to(bias, (128, OUT_C)).astype(np.float32)

    from concourse.bass_utils import run_bass_kernel_spmd

    nc = _build_bass(tiles_total, [int(t) for t in env], sidx_tot,
                     stream_fp16)

    in_maps = [
        {
            "w": np.ascontiguousarray(w_perm[c]),
            "xj": np.ascontiguousarray(xj[c]),
            "sidx": np.ascontiguousarray(sidx[c]),
            "biasb": bias_b,
        }
        for c in range(N_CORES)
    ]

    trace = bool(int(os.environ.get("GNN_TRACE", "0")))
    res = run_bass_kernel_spmd(
        nc, in_maps, core_ids=list(range(N_CORES)), trace=trace)

    _LAST_RUN_INFO.clear()
    _LAST_RUN_INFO.update(
        exec_time_ns=res.exec_time_ns,
        mean_exec_time_ns=res.mean_exec_time_ns,
        tiles_total=tiles_total,
        t_per_blk=float(np.mean(env)),
        profile_json=res.profile_json,
        instructions_and_trace=res.instructions_and_trace,
    )

    # un-permute windows -> blocks, concatenate cores
    outs = []
    for c in range(N_CORES):
        by_win = res.results[c]["out"]                  # [nblk, 128, OUT_C]
        by_blk = np.empty_like(by_win)
        by_blk[order_c[c]] = by_win
        outs.append(by_blk.reshape(nblk * BLK, OUT_C)[:n_per_core])
    out = np.concatenate(outs, axis=0)
    return out[:N]


# revision 30
# speedup vs baseline: 1.4634x; 1.1557x over previous
"""Trainium2 Bass kernel for CustomGraphConv message passing.

reference computation:
    msg  = einsum('eoi,ei->eo', W, x[src])          # per-edge matvec
    aggr = segment_sum(msg, dst, num_segments=N)     # scatter-add
    out  = relu(aggr + bias)

Strategy (8 NeuronCores):
  - Partition OUTPUT nodes across cores: core c owns dst in [c*N/8, (c+1)*N/8).
    Edges are routed to the core owning their destination -> no all-reduce.
  - Within a core, edges are binned by destination block of 128 nodes.
    Blocks are processed as "windows" sorted by descending edge count; each
    sorted position is padded only to the max tile count across the 8 cores
    (SPMD needs one instruction stream), minimizing zero-weight padding.
  - Per 128-edge tile (one edge per SBUF partition):
      S[e, n]     = one-hot of dst_local[e]      (GpSimd local_scatter)
      Q[e, (o,i)] = W[e,o,i] * xj[e,i]           (DVE mult, bf16 out)
      PSUM[n,(o,i)] += S.T @ Q                   (PE matmul, f32 accum)
    After T_w tiles: out[n,o] = relu(sum_i PSUM[n,(o,i)] + bias[o])
  - x[src] is gathered on host and streamed densely (adds ~6% traffic vs W).
  - Queues are single-purpose (sync=W, scalar=xj/sidx/out, Pool=scatter,
    DVE=mult/reduce/bias/relu) and PSUM eviction is software-pipelined two
    windows behind, so no in-order sequencer ever head-of-line blocks a
    prefetch DMA on a drain semaphore.
"""

import os
import sys
import numpy as np

sys.path.insert(0, "/opt/trn_rl_repo")

_LAST_RUN_INFO = {}

N_CORES = 8
BLK = 128          # nodes per destination block (= one-hot window / PSUM rows)
IN_C = 16
OUT_C = 16


def _install_ntff_hook():
    """Provide antenv.axon_hooks if the image lacks it (profiling only)."""
    import types
    import contextlib
    import ctypes

    if "antenv.axon_hooks" in sys.modules:
        return
    try:
        import antenv.axon_hooks  # noqa: F401
        return
    except ImportError:
        pass

    mod = types.ModuleType("antenv.axon_hooks")
    mod._hook = None
    mod._tried = False

    def set_axon_ntff_profile_hook(h):
        mod._hook = h

    def _via_ctypes(so_path):
        lib = ctypes.CDLL(so_path)
        if not hasattr(lib, "axon_start_nrt_profile"):
            return None
        lib.axon_start_nrt_profile.argtypes = [
            ctypes.POINTER(ctypes.c_int64),
            ctypes.c_size_t,
        ]
        lib.axon_start_nrt_profile.restype = ctypes.c_int64
        lib.axon_stop_nrt_profile.argtypes = [ctypes.c_char_p]
        lib.axon_stop_nrt_profile.restype = ctypes.c_int64

        @contextlib.contextmanager
        def _hook_cm(output_dir, device_ids):
            import jax

            jax.devices()
            if device_ids:
                ids = (ctypes.c_int64 * len(device_ids))(*device_ids)
                rc = lib.axon_start_nrt_profile(ids, len(device_ids))
            else:
                rc = lib.axon_start_nrt_profile(None, 0)
            if rc != 0:
                raise RuntimeError(f"axon_start_nrt_profile rc={rc}")
            try:
                yield
            finally:
                n = lib.axon_stop_nrt_profile(str(output_dir).encode())
                print(f"profile: {n} file(s) written to {output_dir}",
                      file=sys.stderr)

        return _hook_cm

    def get_axon_ntff_profile_hook():
        if mod._hook is None and not mod._tried:
            mod._tried = True
            so = os.environ.get("AXON_PJRT_SO", "/opt/axon/libaxon_pjrt.so")
            if os.path.exists(so):
                try:
                    mod._hook = _via_ctypes(so)
                except OSError:
                    mod._hook = None
        return mod._hook

    mod.set_axon_ntff_profile_hook = set_axon_ntff_profile_hook
    mod.get_axon_ntff_profile_hook = get_axon_ntff_profile_hook
    sys.modules["antenv.axon_hooks"] = mod


def _build_bass(tiles_total, env, sidx_tot, stream_fp16):
    import concourse.bacc as bacc
    import concourse.tile as tile
    import concourse.mybir as mybir

    f32 = mybir.dt.float32
    f16 = mybir.dt.bfloat16
    sdt = f16 if stream_fp16 else f32   # dtype of W / xj streams
    nblk = len(env)
    t_max = max(env)
    off = [0]
    for t in env:
        off.append(off[-1] + t)
    so = [0]
    for t in env:
        h = t // 2
        so.append(so[-1] + 2 * (h + h % 2))

    nc = bacc.Bacc("TRN2", target_bir_lowering=False, debug=False,
                   num_devices=N_CORES)

    w_d = nc.dram_tensor("w", [128, tiles_total, 256], sdt,
                         kind="ExternalInput")
    xj_d = nc.dram_tensor("xj", [128, tiles_total, IN_C], sdt,
                          kind="ExternalInput")
    sx_d = nc.dram_tensor("sidx", [128, sidx_tot], mybir.dt.int16,
                          kind="ExternalInput")
    bias_d = nc.dram_tensor("biasb", [128, OUT_C], f32, kind="ExternalInput")
    out_d = nc.dram_tensor("out", [nblk, 128, OUT_C], f32,
                           kind="ExternalOutput")

    # windows are DMA'd in pairs: one large contiguous descriptor per
    # partition per pair (~17KB) minimizes issue count and keeps the DMA
    # queues saturated. Pair sizes (sorted-descending envelope -> first
    # pairs are largest).
    pair_starts = list(range(0, nblk, 2))
    tp_max = max(off[min(p + 2, nblk)] - off[p] for p in pair_starts)
    sx_max = max(so[min(p + 2, nblk)] - so[p] for p in pair_starts)

    with tile.TileContext(nc) as tc:
        with (
            tc.tile_pool(name="wpool", bufs=4) as wpool,
            tc.tile_pool(name="xpool", bufs=4) as xpool,
            tc.tile_pool(name="dpool", bufs=4) as dpool,
            tc.tile_pool(name="spool", bufs=4) as spool,
            tc.tile_pool(name="qpool", bufs=4) as qpool,
            tc.tile_pool(name="opool", bufs=4) as opool,
            tc.tile_pool(name="cpool", bufs=1) as cpool,
            tc.tile_pool(name="psum", bufs=4, space="PSUM") as psum_pool,
        ):
            bias_t = cpool.tile([128, OUT_C], f32, tag="bias")
            nc.sync.dma_start(bias_t[:], bias_d[:])
            ones_t = cpool.tile([128, t_max], f16, tag="ones")
            nc.vector.memset(ones_t[:], 1.0)

            ps_tiles = {}

            def emit_pair(p):
                ws = [w for w in (p, p + 1) if w < nblk]
                base = off[p]
                TP = off[ws[-1] + 1] - base
                sxlen = so[ws[-1] + 1] - so[p]
                wt = wpool.tile([128, tp_max, 256], sdt, tag="wt")
                nc.sync.dma_start(wt[:, :TP, :], w_d[:, base:base + TP, :])
                xt = xpool.tile([128, tp_max, IN_C], sdt, tag="xt")
                nc.scalar.dma_start(xt[:, :TP, :], xj_d[:, base:base + TP, :])
                sx = dpool.tile([128, sx_max], mybir.dt.int16, tag="sx")
                nc.scalar.dma_start(sx[:, :sxlen], sx_d[:, so[p]:so[p] + sxlen])

                for w in ws:
                    T = env[w]
                    H = T // 2
                    NI = H + (H % 2)
                    wo = off[w] - base          # tile offset inside pair
                    sxo = so[w] - so[p]
                    ps = psum_pool.tile([128, 256], f32)
                    ps_tiles[w] = ps
                    st = spool.tile([128, t_max, BLK], f16, tag="st")
                    qt = qpool.tile([128, t_max, OUT_C, IN_C], f16, tag="qt")
                    for h in range(2):
                        lo, hi = h * H, (h + 1) * H
                        # one-hot S (GpSimd: zero + scatter ones)
                        nc.gpsimd.local_scatter(
                            st[:, lo:hi, :].rearrange("p t n -> p (t n)"),
                            ones_t[:, :NI],
                            sx[:, sxo + h * NI:sxo + (h + 1) * NI],
                            channels=128,
                            num_elems=H * BLK,
                            num_idxs=NI,
                        )
                        # per-edge products
                        nc.vector.tensor_tensor(
                            qt[:, lo:hi, :, :],
                            wt[:, wo + lo:wo + hi, :].rearrange(
                                "p g (o i) -> p g o i", i=IN_C),
                            xt[:, wo + lo:wo + hi, :].unsqueeze(2)
                                .broadcast_to([128, H, OUT_C, IN_C]),
                            op=mybir.AluOpType.mult,
                        )
                    for k in range(T):
                        nc.tensor.matmul(
                            ps[:],
                            st[:, k, :],
                            qt[:, k, :, :],
                            start=(k == 0),
                            stop=(k == T - 1),
                        )

            def emit_back(b):
                ps = ps_tiles.pop(b)
                ot = opool.tile([128, OUT_C], f32, tag="ot")
                nc.vector.tensor_reduce(
                    ot[:],
                    ps[:].rearrange("p (o i) -> p o i", i=IN_C),
                    axis=mybir.AxisListType.X,
                    op=mybir.AluOpType.add,
                )
                ob = opool.tile([128, OUT_C], f32, tag="ob")
                nc.vector.tensor_tensor(
                    ob[:], ot[:], bias_t[:], op=mybir.AluOpType.add)
                orl = opool.tile([128, OUT_C], f32, tag="orl")
                nc.vector.tensor_relu(orl[:], ob[:])
                nc.scalar.dma_start(out_d[b], orl[:])

            # software pipeline: a window's PSUM eviction is emitted one
            # pair later so in-order engine queues never stall on the PE
            # finishing the current window.
            for p in pair_starts:
                emit_pair(p)
                for w in (p - 2, p - 1):
                    if 0 <= w < nblk:
                        emit_back(w)
            for w in (nblk - 2, nblk - 1):
                if 0 <= w < nblk:
                    emit_back(w)

    nc.compile()
    return nc


def kernel(x, edge_index, edge_attr, weights_matrices, bias,
           input_size, output_size, **_unused):
    _install_ntff_hook()
    import ml_dtypes

    stream_fp16 = bool(int(os.environ.get("GNN_STREAM_FP16", "1")))
    sdt_np = ml_dtypes.bfloat16 if stream_fp16 else np.float32

    x = np.asarray(x, dtype=np.float32)
    edge_index = np.asarray(edge_index)
    W = np.asarray(weights_matrices, dtype=np.float32)
    bias = np.asarray(bias, dtype=np.float32)

    N = x.shape[0]
    E = edge_index.shape[1]
    n_per_core = (N + N_CORES - 1) // N_CORES          # 12500
    nblk = (n_per_core + BLK - 1) // BLK               # 98

    src = edge_index[0].astype(np.int64)
    dst = edge_index[1].astype(np.int64)

    core = dst // n_per_core
    local = dst - core * n_per_core
    blk = local // BLK
    dstl = (local - blk * BLK).astype(np.int64)         # in [0,128)

    # group edges by (core, block)
    key = core * nblk + blk
    order = np.argsort(key, kind="stable")
    key_sorted = key[order]
    counts = np.bincount(key_sorted, minlength=N_CORES * nblk)
    t_cb = (counts.reshape(N_CORES, nblk) + BLK - 1) // BLK
    t_cb = np.maximum(t_cb, 1)

    # windows: per core, blocks sorted by descending tile count; pad each
    # sorted position to the max across cores (one SPMD instruction stream)
    order_c = np.argsort(-t_cb, axis=1, kind="stable")  # [cores, nblk]
    t_sorted = np.take_along_axis(t_cb, order_c, axis=1)
    env = t_sorted.max(axis=0).astype(np.int64)
    env += env % 2                                      # even (scatter halves)
    off = np.zeros(nblk + 1, np.int64)
    np.cumsum(env, out=off[1:])
    tiles_total = int(off[-1])
    epc = tiles_total * BLK                             # padded edges per core
    H_w = env // 2
    NI_w = H_w + (H_w % 2)
    so = np.zeros(nblk + 1, np.int64)
    np.cumsum(2 * NI_w, out=so[1:])
    sidx_tot = int(so[-1])
    win_cb = np.empty_like(order_c)
    win_cb[np.arange(N_CORES)[:, None], order_c] = np.arange(nblk)[None, :]

    # slot position of each sorted edge inside its (core, window) bucket
    group_start = np.zeros(N_CORES * nblk + 1, dtype=np.int64)
    np.cumsum(counts, out=group_start[1:])
    within = np.arange(E, dtype=np.int64) - group_start[key_sorted]
    core_s = key_sorted // nblk
    blk_s = key_sorted - core_s * nblk
    win_s = win_cb[core_s, blk_s]
    pos = off[win_s] * BLK + within                     # slot within core

    # perm[c, slot] = original edge id, -1 for padding
    perm = np.full((N_CORES, epc), -1, dtype=np.int64)
    perm[core_s, pos] = order

    pad_mask = perm < 0
    perm_c = np.where(pad_mask, 0, perm)

    # per-core streams; layout [cores, 128 partitions, tiles, ...]
    # edge slot s -> tile s // 128, partition s % 128
    def to_tiles(a):
        F = a.shape[-1]
        return np.ascontiguousarray(
            a.reshape(N_CORES, tiles_total, BLK, F).transpose(0, 2, 1, 3))

    Wf = W.reshape(E, IN_C * OUT_C)
    w_perm = Wf[perm_c].astype(sdt_np)
    w_perm[pad_mask] = 0.0
    w_perm = to_tiles(w_perm)

    xj = x[src[perm_c]].astype(sdt_np)
    xj = to_tiles(xj)

    # scatter indices for the one-hot build: edge slot s -> partition s%128,
    # tile s//128 -> window w, tile-in-window k, half h = k//H_w,
    # sidx column so[w] + h*NI_w + (k%H_w), value (k%H_w)*BLK + dst_local
    s_arr = np.arange(epc, dtype=np.int64)
    p_arr = s_arr % BLK
    tile_arr = s_arr // BLK
    w_arr = np.searchsorted(off, tile_arr, side="right") - 1
    k_arr = tile_arr - off[w_arr]
    h_arr = k_arr // H_w[w_arr]
    kih_arr = k_arr - h_arr * H_w[w_arr]
    col_arr = so[w_arr] + h_arr * NI_w[w_arr] + kih_arr
    dl_perm = dstl[perm_c]                              # [cores, epc]
    val = (kih_arr[None, :] * BLK + dl_perm).astype(np.int16)
    val[pad_mask] = -1
    sidx = np.full((N_CORES, 128, sidx_tot), -1, dtype=np.int16)
    c_idx = np.repeat(np.arange(N_CORES), epc)
    sidx[c_idx, np.tile(p_arr, N_CORES), np.tile(col_arr, N_CORES)] = \
        val.ravel()

    bias_b = np.broadcast_to(bias, (128, OUT_C)).astype(np.float32)

    from concourse.bass_utils import run_bass_kernel_spmd

    nc = _build_bass(tiles_total, [int(t) for t in env], sidx_tot,
                     stream_fp16)

    in_maps = [
        {
            "w": np.ascontiguousarray(w_perm[c]),
            "xj": np.ascontiguousarray(xj[c]),
            "sidx": np.ascontiguousarray(sidx[c]),
            "biasb": bias_b,
        }
        for c in range(N_CORES)
    ]

    trace = bool(int(os.environ.get("GNN_TRACE", "0")))
    res = run_bass_kernel_spmd(
        nc, in_maps, core_ids=list(range(N_CORES)), trace=trace)

    _LAST_RUN_INFO.clear()
    _LAST_RUN_INFO.update(
        exec_time_ns=res.exec_time_ns,
        mean_exec_time_ns=res.mean_exec_time_ns,
        tiles_total=tiles_total,
        t_per_blk=float(np.mean(env)),
        profile_json=res.profile_json,
        instructions_and_trace=res.instructions_and_trace,
    )

    # un-permute windows -> blocks, concatenate cores
    outs = []
    for c in range(N_CORES):
        by_win = res.results[c]["out"]                  # [nblk, 128, OUT_C]
        by_blk = np.empty_like(by_win)
        by_blk[order_c[c]] = by_win
        outs.append(by_blk.reshape(nblk * BLK, OUT_C)[:n_per_core])
    out = np.concatenate(outs, axis=0)
    return out[:N]
